# revision 51
# baseline (speedup 1.0000x reference)
"""BiLSTM-CRF loss kernel for 8 Trainium2 NeuronCores (data-parallel over batch).

Self-contained: hardcodes all shapes from the problem spec.
Returns scalar f32 loss (mean over batch of CRF NLL).

Math reformulation (validated vs reference):
 - LSTM gates via one tanh (sigmoid(x) = 0.5 tanh(x/2) + 0.5); i,f,o weight
   rows pre-halved on host. States kept as S = 2c, h' = 2h (weights absorb).
   Cell: u=(th_i+1)*th_g; w=(th_f+1)*S; S'=0.5w+u; h'=(th_o+1)*tanh(S'/2).
 - Reverse-direction masking: add -30000 to i,f,o pre-activations at padded
   steps (sigmoids -> 0 => state resets). Forward needs no masking.
 - Time-split chains: each direction's scan is split in two halves run as
   independent chains; the second half starts KW steps early from zero state
   (forget-gate decay makes the state converge; exact when the split point
   falls in padding).
 - Embedding gather via gpsimd dma_gather on a host-compacted (np.unique)
   int16-indexed table, rows padded to 128 (bias 1.0 planted at col 50),
   PE-transposed into xT.
 - LayerNorm folded into the feature matmul; mu/var from PE ones-matmul
   reductions; packing via DMA reshape.  Feats phase processed in
   super-chunks of 4x32 K-rows stacked on 128 partitions.
 - CRF in exp space: w_t = exp(alpha_t) * kappa^t with kappa folded into the
   transition matrix. alpha-at-sen_len recovered from the END-transition
   readout row, staged per step, gathered at sen_len.
 - Gold-path transition/END/beta terms are pure functions of the inputs and
   are folded into a per-sequence host constant; device computes only the
   emission sum.
"""

import numpy as np
import ml_dtypes

VOCAB, EMBD, HID, K = 100000, 50, 200, 32
H = 100
START, END = 30, 31
B, T = 512, 256
NCORES = 8
BC = B // NCORES            # 64 sequences per core
LN_EPS = 1e-5
KLOG = 4.9                  # -log(kappa)
KW = 24                     # warmup steps for second-half chains
VC = 16384                  # compacted vocab rows (>= distinct tokens/core)
EROW = 128                  # padded embedding row (bf16, 256B)

bf16 = ml_dtypes.bfloat16

_PROGRAM_CACHE = {}
DEBUG_DUMP = False


def _chain_defs(Tn):
    """Four LSTM chains (uniform length Tn//2 + KW/... ): forward split at FS,
    backward split at BS, so every chain runs exactly Tn//2 + ... slots."""
    FS = Tn // 2 + KW // 2          # 140
    BS = Tn // 2 - KW // 2          # 116
    f0 = ("f0", "f", list(range(0, FS)))                 # stores 0..FS-1
    f1 = ("f1", "f", list(range(FS - KW, Tn)))           # stores FS..Tn-1
    b0 = ("b0", "b", list(range(Tn - 1, BS - 1, -1)))    # stores BS..Tn-1
    b1 = ("b1", "b", list(range(BS - 1 + KW, -1, -1)))   # stores 0..BS-1
    chains = []
    for name, dn, ts in (f0, b0, f1, b1):
        if name == "f0":
            stored = lambda t: t < FS
        elif name == "f1":
            stored = lambda t: t >= FS
        elif name == "b0":
            stored = lambda t: t >= BS
        else:
            stored = lambda t: t < BS
        chains.append(dict(name=name, dn=dn, ts=ts, stored=stored))
    return chains


def _gather_order(Tn):
    """Order of 1024-token gather chunks (16 t-steps each) by first need."""
    FS = Tn // 2 + KW // 2
    BS = Tn // 2 - KW // 2
    ngc = Tn // 16
    def first_need(g):
        t0, t1 = 16 * g, 16 * g + 15
        cands = []
        if t0 < FS:
            cands.append(t0)                            # f0 at slot t
        if t1 >= FS - KW:
            cands.append(max(0, t0 - (FS - KW)))        # f1
        if t1 >= BS:
            cands.append(Tn - 1 - t1)                   # b0
        if t0 < BS + KW:
            cands.append(max(0, (BS - 1 + KW) - t1))    # b1
        return min(cands)
    return sorted(range(ngc), key=first_need)


def _pass1_ready(Tn):
    """Per 512-token chunk (8 t), slot at which both hf and hb are stored."""
    FS = Tn // 2 + KW // 2
    BS = Tn // 2 - KW // 2
    def ready(c):
        r = 0
        for t in range(8 * c, 8 * c + 8):
            fr = t if t < FS else t - (FS - KW)
            br = (Tn - 1 - t) if t >= BS else (BS - 1 + KW) - t
            r = max(r, fr, br)
        return r
    return [ready(c) for c in range(Tn * BC // 512)]


def _build_program(Tn):
    import concourse.bass as bass
    import concourse.bacc as bacc
    import concourse.mybir as mybir
    import concourse.tile as tile
    from concourse.alu_op_type import AluOpType as op
    from concourse.masks import make_identity
    from concourse.library_config import mlp
    from contextlib import ExitStack

    dt = mybir.dt
    AF = mybir.ActivationFunctionType
    NT = Tn * BC                 # 16384 tokens
    NGC = NT // 1024             # gather chunks
    NCH = NT // 512              # feat chunks (8 t each)
    NSC = NCH // 4               # super-chunks (32 t each)
    PKC = NT // 128              # packed free size (128)
    half = Tn // 2
    NOCT = (Tn + 1 + 7) // 8

    chains = _chain_defs(Tn)
    gorder = _gather_order(Tn)
    p1ready = _pass1_ready(Tn)
    nslots = max(len(c["ts"]) for c in chains)

    nc = bacc.Bacc()

    d_embc = nc.dram_tensor("embc", [VC, EROW], dt.bfloat16, kind="ExternalInput")
    d_gidx = nc.dram_tensor("gidx", [128, NT // 16], dt.int16, kind="ExternalInput")
    d_invm = nc.dram_tensor("invm", [1, NT], dt.bfloat16, kind="ExternalInput")
    d_wx = {dn: nc.dram_tensor(f"wx_{dn}", [EMBD + 2, 4 * H], dt.bfloat16,
                               kind="ExternalInput") for dn in "fb"}
    d_wh = {dn: nc.dram_tensor(f"wh_{dn}", [H, 4 * H], dt.bfloat16,
                               kind="ExternalInput") for dn in "fb"}
    d_wgf = nc.dram_tensor("wgt_f", [H, K], dt.bfloat16, kind="ExternalInput")
    d_wgb = nc.dram_tensor("wgt_b", [H, K], dt.bfloat16, kind="ExternalInput")
    d_nws = nc.dram_tensor("negwsum", [1, K], dt.bfloat16, kind="ExternalInput")
    d_c0 = nc.dram_tensor("c0col", [K, 1], dt.float32, kind="ExternalInput")
    d_ohem = nc.dram_tensor("ohem", [K, NT], dt.bfloat16, kind="ExternalInput")
    d_mmat = nc.dram_tensor("mmat", [K, K + 1], dt.bfloat16, kind="ExternalInput")
    d_w0 = nc.dram_tensor("w0", [K, BC], dt.bfloat16, kind="ExternalInput")
    d_ui = nc.dram_tensor("u_idx", [BC, 1], dt.int32, kind="ExternalInput")
    d_lenk = nc.dram_tensor("len_klog", [BC, 1], dt.float32, kind="ExternalInput")
    d_loss = nc.dram_tensor("loss", [BC, 1], dt.float32, kind="ExternalOutput")
    if DEBUG_DUMP:
        d_dbg = dict(
            hf=nc.dram_tensor("dbg_hf", [H, NT], dt.bfloat16, kind="ExternalOutput"),
            hb=nc.dram_tensor("dbg_hb", [H, NT], dt.bfloat16, kind="ExternalOutput"),
            xt=nc.dram_tensor("dbg_xt", [EMBD + 2, NT], dt.bfloat16, kind="ExternalOutput"),
            epk=nc.dram_tensor("dbg_epk", [K, NT], dt.bfloat16, kind="ExternalOutput"),
            mu=nc.dram_tensor("dbg_mu", [16, NT // 16], dt.float32, kind="ExternalOutput"),
            msq=nc.dram_tensor("dbg_msq", [16, NT // 16], dt.float32, kind="ExternalOutput"),
            rstd=nc.dram_tensor("dbg_rstd", [16, NT // 16], dt.bfloat16, kind="ExternalOutput"),
            rsub=nc.dram_tensor("dbg_rsub", [1, BC], dt.float32, kind="ExternalOutput"),
            ud=nc.dram_tensor("dbg_ud", [NOCT * 8 * BC, 1], dt.float32, kind="ExternalOutput"),
            ug=nc.dram_tensor("dbg_ug", [BC, 1], dt.float32, kind="ExternalOutput"),
        )

    with tile.TileContext(nc) as tc, ExitStack() as ctx:
        const = ctx.enter_context(tc.tile_pool(name="const", bufs=1))
        big = ctx.enter_context(tc.tile_pool(name="big", bufs=1))
        dramp = ctx.enter_context(tc.tile_pool(name="dramp", bufs=1, space="DRAM"))

        u_d = dramp.tile([NOCT * 8 * BC, 1], dt.float32, tag="u_d")
        r_d = dramp.tile([BC, 1], dt.float32, tag="r_d")

        nc.gpsimd.load_library(mlp)

        ident = const.tile([128, 128], dt.bfloat16)
        make_identity(nc, ident[:])
        wx = {dn: const.tile([EMBD + 2, 4 * H], dt.bfloat16, tag=f"wx{dn}", name=f"wx{dn}") for dn in "fb"}
        wh = {dn: const.tile([H, 4 * H], dt.bfloat16, tag=f"wh{dn}", name=f"wh{dn}") for dn in "fb"}
        for dn in "fb":
            nc.sync.dma_start(wx[dn][:], d_wx[dn][:])
            nc.sync.dma_start(wh[dn][:], d_wh[dn][:])
        wgf = const.tile([H, K], dt.bfloat16)
        nc.sync.dma_start(wgf[:], d_wgf[:])
        wgb = const.tile([H, K], dt.bfloat16)
        nc.sync.dma_start(wgb[:], d_wgb[:])
        nws = const.tile([1, K], dt.bfloat16)
        nc.sync.dma_start(nws[:], d_nws[:])
        c0col = const.tile([K, 1], dt.float32)
        nc.sync.dma_start(c0col[:], d_c0[:])
        ohem = const.tile([K, NT], dt.bfloat16, tag="ohem")
        nc.sync.dma_start(ohem[:], d_ohem[:])
        ones1k = const.tile([1, K], dt.bfloat16)
        nc.vector.memset(ones1k[:], 1.0)
        ones1kf = const.tile([K, 1], dt.bfloat16)
        nc.vector.memset(ones1kf[:], 1.0)
        mm_t = const.tile([K, K + 1], dt.bfloat16)
        nc.sync.dma_start(mm_t[:], d_mmat[:])
        w0t = const.tile([K, BC], dt.bfloat16)
        nc.sync.dma_start(w0t[:], d_w0[:])
        gidx = const.tile([128, NT // 16], dt.int16, tag="gidx")
        nc.sync.dma_start(gidx[:], d_gidx[:])
        ones100 = const.tile([H, 1], dt.bfloat16)
        nc.vector.memset(ones100[:], 1.0)
        ones128 = const.tile([128, 1], dt.bfloat16)
        nc.vector.memset(ones128[:], 1.0)

        xT = big.tile([128, NT], dt.bfloat16, tag="xT")
        hq = {dn: [big.tile([H, 64 * BC], dt.bfloat16, tag=f"h{dn}{q}", name=f"h{dn}{q}")
                   for q in range(Tn // 64)] for dn in "fb"}
        epkK = big.tile([K, NT], dt.bfloat16, tag="epkK")
        mspk = big.tile([128, 2 * PKC], dt.float32, tag="mspk")
        warm = {c["name"]: big.tile([H, 2 * BC], dt.bfloat16, tag=f"wm{c['name']}", name=f"wm{c['name']}")
                for c in chains if c["name"] in ("f1", "b1")}
        S = {c["name"]: [big.tile([H, BC], dt.float32, tag=f"S{c['name']}{j}", name=f"S{c['name']}{j}")
                         for j in range(2)] for c in chains}
        for c in chains:
            nc.vector.memset(S[c["name"]][0][:], 0.0)

        # ================ P0: embedding gather + transpose ================
        ctx0 = ExitStack()
        p0 = ctx0.enter_context(tc.tile_pool(name="p0", bufs=3))
        p0ps = ctx0.enter_context(tc.tile_pool(name="p0ps", bufs=1, space="PSUM"))
        for gi, g in enumerate(gorder):
            xg = p0.tile([128, 8 * EROW], dt.bfloat16, tag="xg")
            nc.gpsimd.dma_gather(
                xg[:].rearrange("p (k e) -> p k e", e=EROW),
                d_embc[:],
                gidx[:, g * 64:(g + 1) * 64],
                1024, 1024, EROW,
            )
            for hf in range(2):
                tp = p0ps.tile([EMBD + 2, 512], dt.bfloat16, tag="tp")
                for j in range(4):
                    sl = hf * 4 + j
                    nc.tensor.matmul(
                        out=tp[:, j * 128:(j + 1) * 128],
                        lhsT=xg[:, sl * EROW:sl * EROW + EMBD + 2],
                        rhs=ident[:], is_transpose=True,
                        start=(j == 0), stop=(j == 3),
                    )
                dst = g * 1024 + hf * 512
                nc.vector.tensor_copy(out=xT[0:EMBD + 2, dst:dst + 512], in_=tp[:])
            nc.sync.dma_start(out=xT[EMBD + 1:EMBD + 2, g * 1024:(g + 1) * 1024],
                              in_=d_invm[:, g * 1024:(g + 1) * 1024])
        ctx0.close()

        # ================ P1: four LSTM chains + interleaved pass1/pass2 ==
        goldps = ctx.enter_context(tc.tile_pool(name="goldps", bufs=1, space="PSUM"))
        realp = goldps.tile([1, 512], dt.float32, tag="realp")
        ctx1 = ExitStack()
        p1 = ctx1.enter_context(tc.tile_pool(name="p1", bufs=2))
        p1psA = ctx1.enter_context(tc.tile_pool(name="p1psA", bufs=1, space="PSUM"))
        p1psB = ctx1.enter_context(tc.tile_pool(name="p1psB", bufs=1, space="PSUM"))
        p2a = ctx1.enter_context(tc.tile_pool(name="p2a", bufs=2))
        p2aps = ctx1.enter_context(tc.tile_pool(name="p2aps", bufs=1, space="PSUM"))
        p2 = ctx1.enter_context(tc.tile_pool(name="p2", bufs=3))
        p2s = ctx1.enter_context(tc.tile_pool(name="p2s", bufs=1))
        p2ps = ctx1.enter_context(tc.tile_pool(name="p2ps", bufs=1, space="PSUM"))
        packRS = p2s.tile([128, 2 * PKC], dt.bfloat16, tag="packRS")
        sq = p2s.tile([128, PKC], dt.float32, tag="sqpk")
        var = p2s.tile([128, PKC], dt.float32, tag="varpk")
        lnv = p2s.tile([128, PKC], dt.float32, tag="lnvpk")
        epsc = p2s.tile([128, 1], dt.float32, tag="epsc")
        nc.vector.memset(epsc[:], LN_EPS)

        def h_ap(c, j):
            """AP where chain c's step-j h output lives."""
            t = c["ts"][j]
            if c["stored"](t):
                return hq[c["dn"]][t // 64][:, (t % 64) * BC:(t % 64 + 1) * BC]
            return warm[c["name"]][:, (j % 2) * BC:(j % 2 + 1) * BC]

        p1_stash = {}

        def emit_pass1(ch):
            q, off = (ch * 512) // (64 * BC), (ch * 512) % (64 * BC)
            hfc = hq["f"][q][:, off:off + 512]
            hbc = hq["b"][q][:, off:off + 512]
            hsqf = p2a.tile([H, 512], dt.bfloat16, tag="hsqf")
            nc.gpsimd.tensor_tensor(out=hsqf[:], in0=hfc, in1=hfc, op=op.mult)
            hsqb = p2a.tile([H, 512], dt.bfloat16, tag="hsqb")
            nc.gpsimd.tensor_tensor(out=hsqb[:], in0=hbc, in1=hbc, op=op.mult)
            stgb = p2a.tile([1, 1024], dt.float32, tag="stgb")
            psmu = p2aps.tile([1, 512], dt.float32, tag="psmu")
            nc.tensor.matmul(out=psmu[:], lhsT=ones100[:], rhs=hfc, start=True, stop=False)
            nc.tensor.matmul(out=psmu[:], lhsT=ones100[:], rhs=hbc, start=False, stop=True)
            nc.vector.tensor_copy(out=stgb[:, 0:512], in_=psmu[:])
            psmsq = p2aps.tile([1, 512], dt.float32, tag="psmu")
            nc.tensor.matmul(out=psmsq[:], lhsT=ones100[:], rhs=hsqf[:], start=True, stop=False)
            nc.tensor.matmul(out=psmsq[:], lhsT=ones100[:], rhs=hsqb[:], start=False, stop=True)
            nc.vector.tensor_copy(out=stgb[:, 512:1024], in_=psmsq[:])
            nc.sync.dma_start(out=mspk[4 * ch:4 * ch + 4, 0:PKC],
                              in_=stgb[:, 0:512])
            nc.sync.dma_start(out=mspk[4 * ch:4 * ch + 4, PKC:2 * PKC],
                              in_=stgb[:, 512:1024])

        def emit_rstd(g):
            r = slice(32 * g, 32 * g + 32)
            nc.vector.scalar_tensor_tensor(out=sq[r, :], in0=mspk[r, 0:PKC],
                                           scalar=1.0 / 160000.0,
                                           in1=mspk[r, 0:PKC], op0=op.mult, op1=op.mult)
            nc.vector.scalar_tensor_tensor(out=var[r, :], in0=mspk[r, PKC:2 * PKC],
                                           scalar=1.0 / 800.0,
                                           in1=sq[r, :], op0=op.mult, op1=op.subtract)
            nc.scalar.activation(out=lnv[r, :], in_=var[r, :], func=AF.Ln,
                                 bias=epsc[r, :])
            nc.scalar.activation(out=packRS[r, 0:PKC], in_=lnv[r, :], func=AF.Exp,
                                 scale=-0.5)
            nc.vector.tensor_copy(out=packRS[r, PKC:2 * PKC], in_=mspk[r, 0:PKC])

        n_p2 = [0]

        def emit_pass2(ch):
            q, off = (ch * 512) // (64 * BC), (ch * 512) % (64 * BC)
            pg = p2ps.tile([K, 512], dt.float32, tag="pg")
            uboth = p2.tile([1, 1024], dt.bfloat16, tag="uboth")
            nc.sync.dma_start(out=uboth[:, 0:512],
                              in_=packRS[4 * ch:4 * ch + 4, 0:PKC])
            nc.sync.dma_start(out=uboth[:, 512:1024],
                              in_=packRS[4 * ch:4 * ch + 4, PKC:2 * PKC])
            nc.tensor.matmul(out=pg[:], lhsT=wgf[:], rhs=hq["f"][q][:, off:off + 512],
                             start=True, stop=False)
            nc.tensor.matmul(out=pg[:], lhsT=wgb[:], rhs=hq["b"][q][:, off:off + 512],
                             start=False, stop=False)
            nc.tensor.matmul(out=pg[:], lhsT=nws[:], rhs=uboth[:, 512:1024],
                             start=False, stop=True)
            rb = p2ps.tile([K, 512], dt.float32, tag="rb")
            nc.tensor.matmul(out=rb[:], lhsT=ones1k[:], rhs=uboth[:, 0:512],
                             start=True, stop=True)
            rbs = p2.tile([K, 512], dt.bfloat16, tag="rbs")
            nc.scalar.copy(out=rbs[:], in_=rb[:])
            fsl = p2.tile([K, 512], dt.bfloat16, tag="fsl")
            nc.vector.tensor_tensor(out=fsl[:], in0=pg[:], in1=rbs[:], op=op.mult)
            nc.scalar.activation(out=epkK[:, ch * 512:(ch + 1) * 512], in_=fsl[:],
                                 func=AF.Exp, bias=c0col[:])
            esel = p2.tile([K, 512], dt.bfloat16, tag="esel")
            nc.vector.tensor_tensor(out=esel[:], in0=fsl[:],
                                    in1=ohem[:, ch * 512:(ch + 1) * 512],
                                    op=op.mult)
            k = n_p2[0]
            nc.tensor.matmul(out=realp[:], lhsT=ones1kf[:], rhs=esel[:],
                             start=(k == 0), stop=(k == NCH - 1))
            n_p2[0] += 1

        grp_ready = [max(p1ready[8 * g:8 * g + 8]) for g in range(NCH // 8)]
        pending1 = {s: [c for c in range(NCH) if p1ready[c] == s] for s in range(nslots)}
        pending2 = {s: [g for g in range(NCH // 8) if grp_ready[g] == s]
                    for s in range(nslots)}

        for s in range(nslots):
            act_cs = [c for c in chains if s < len(c["ts"])]
            step = {}
            for c in act_cs:
                nm, dn = c["name"], c["dn"]
                t = c["ts"][s]
                first = (s == 0)
                pspool = p1psB if nm in ("f1", "b1") else p1psA
                ps = pspool.tile([H, 4 * BC], dt.float32, tag=f"ps{nm}", name=f"ps{nm}")
                n_mm = 4 if first else 8
                k_mm = 0
                rx = xT[0:EMBD + 2, t * BC:(t + 1) * BC]
                for g in range(4):
                    nc.tensor.matmul(out=ps[:, g * BC:(g + 1) * BC],
                                     lhsT=wx[dn][:, g * H:(g + 1) * H], rhs=rx,
                                     start=(k_mm == 0), stop=(k_mm == n_mm - 1))
                    k_mm += 1
                if not first:
                    rh = h_ap(c, s - 1)
                    for g in range(4):
                        nc.tensor.matmul(out=ps[:, g * BC:(g + 1) * BC],
                                         lhsT=wh[dn][:, g * H:(g + 1) * H], rhs=rh,
                                         start=(k_mm == 0), stop=(k_mm == n_mm - 1))
                        k_mm += 1
                step[nm] = ps
            for c in act_cs:
                nm = c["name"]
                G = p1.tile([H, 4 * BC], dt.bfloat16, tag=f"G{nm}", name=f"G{nm}")
                nc.scalar.activation(out=G[:], in_=step[nm][:], func=AF.Tanh)
                step[nm] = G
            for c in act_cs:
                nm = c["name"]
                G = step[nm]
                u = p1.tile([H, BC], dt.bfloat16, tag=f"u{nm}", name=f"u{nm}")
                nc.vector.scalar_tensor_tensor(out=u[:], in0=G[:, 0:BC], scalar=1.0,
                                               in1=G[:, 3 * BC:4 * BC],
                                               op0=op.add, op1=op.mult)
                w = p1.tile([H, BC], dt.float32, tag=f"w{nm}", name=f"w{nm}")
                nc.vector.scalar_tensor_tensor(out=w[:], in0=G[:, BC:2 * BC], scalar=1.0,
                                               in1=S[nm][s % 2][:],
                                               op0=op.add, op1=op.mult)
                nc.vector.scalar_tensor_tensor(out=S[nm][(s + 1) % 2][:], in0=w[:],
                                               scalar=0.5, in1=u[:],
                                               op0=op.mult, op1=op.add)
            thcs = {}
            for c in act_cs:
                nm = c["name"]
                thc = p1.tile([H, BC], dt.bfloat16, tag=f"thc{nm}", name=f"thc{nm}")
                nc.scalar.activation(out=thc[:], in_=S[nm][(s + 1) % 2][:],
                                     func=AF.Tanh, scale=0.5)
                thcs[nm] = thc
            for c in act_cs:
                nm = c["name"]
                nc.vector.scalar_tensor_tensor(out=h_ap(c, s),
                                               in0=step[nm][:, 2 * BC:3 * BC],
                                               scalar=1.0, in1=thcs[nm][:],
                                               op0=op.add, op1=op.mult)
            for ch in pending1.get(s, []):
                emit_pass1(ch)
            for g in pending2.get(s, []):
                emit_rstd(g)
                for ch in range(8 * g, 8 * g + 8):
                    emit_pass2(ch)
        if DEBUG_DUMP:
            nc.sync.dma_start(d_dbg["rstd"][:], packRS[:, 0:PKC])
        ctx1.close()

        # ================ P3: CRF recursion ===============================
        with tc.tile_pool(name="p3", bufs=2) as p3, \
             tc.tile_pool(name="p3ps", bufs=4, space="PSUM") as p3ps:
            wcur = w0t
            for o in range(NOCT):
                t0, t1 = o * 8 + 1, min(o * 8 + 8, Tn + 1)
                nsteps = t1 - t0 + 1
                pvo = p3ps.tile([K + 1, 512], dt.float32, tag="pvo")
                ustg = p3.tile([K + 1, 512], dt.float32, tag="ustg")
                for t in range(t0, t1 + 1):
                    so = (t - 1) % 8
                    pv = pvo[:, so * BC:(so + 1) * BC]
                    nc.tensor.matmul(out=pv[:], lhsT=mm_t[:],
                                     rhs=wcur[:], start=True, stop=True)
                    if t <= Tn:
                        tok = t - 1
                        wn = p3.tile([K, BC], dt.bfloat16, tag="wn")
                        nc.vector.tensor_tensor(
                            out=wn[:], in0=pv[0:K, :],
                            in1=epkK[:, tok * BC:(tok + 1) * BC],
                            op=op.mult)
                        wcur = wn
                nc.scalar.copy(out=ustg[K:K + 1, :nsteps * BC],
                               in_=pvo[K:K + 1, :nsteps * BC])
                nc.sync.dma_start(
                    out=u_d[(t0 - 1) * BC:(t0 - 1) * BC + nsteps * BC, :],
                    in_=ustg[K:K + 1, :nsteps * BC])

        # ================ P4: final loss ==================================
        with tc.tile_pool(name="p4", bufs=1) as p4:
            rsub = p4.tile([1, BC], dt.float32, tag="rsub")
            nc.vector.tensor_reduce(
                out=rsub[:], in_=realp[:].rearrange("one (t b) -> one b t", b=BC),
                axis=mybir.AxisListType.X, op=op.add)
            if DEBUG_DUMP:
                nc.sync.dma_start(d_dbg["rsub"][:], rsub[:])
            nc.sync.dma_start(out=r_d[:], in_=rsub[:])
            rcol = p4.tile([BC, 1], dt.float32, tag="rcol")
            nc.sync.dma_start(out=rcol[:], in_=r_d[:])

            ui = p4.tile([BC, 1], dt.int32, tag="ui")
            nc.sync.dma_start(ui[:], d_ui[:])
            lenk = p4.tile([BC, 1], dt.float32, tag="lenk")
            nc.sync.dma_start(lenk[:], d_lenk[:])
            ug = p4.tile([BC, 1], dt.float32, tag="ug")
            nc.gpsimd.indirect_dma_start(out=ug[:], out_offset=None, in_=u_d[:],
                                         in_offset=bass.IndirectOffsetOnAxis(ap=ui[:], axis=0))
            tot = p4.tile([BC, 1], dt.float32, tag="tot")
            if DEBUG_DUMP:
                nc.sync.dma_start(d_dbg["ug"][:], ug[:])
            nc.scalar.activation(out=tot[:], in_=ug[:], func=AF.Ln)
            nc.vector.tensor_tensor(out=tot[:], in0=tot[:], in1=lenk[:], op=op.add)
            lout = p4.tile([BC, 1], dt.float32, tag="lout")
            nc.vector.tensor_tensor(out=lout[:], in0=tot[:], in1=rcol[:], op=op.subtract)
            nc.sync.dma_start(out=d_loss[:], in_=lout[:])

    nc.compile()
    return nc


def _prep_consts(emb, Wf_ih, Wf_hh, bfv, Wb_ih, Wb_hh, bbv, gamma, beta, W_lin, trans, Tn):
    sc = np.ones((4 * H, 1), np.float32)
    sc[0:H] = 0.5
    sc[H:2 * H] = 0.5
    sc[3 * H:4 * H] = 0.5
    # reference gate order [i,f,g,o] -> device order [i,f,o,g]
    perm = np.concatenate([np.arange(0, H), np.arange(H, 2 * H),
                           np.arange(3 * H, 4 * H), np.arange(2 * H, 3 * H)])

    def mk(Wi, Wh, b, bwd):
        Wi_s, Wh_s, b_s = Wi * sc, Wh * sc * 0.5, b * sc[:, 0]
        Wi_p, Wh_p, b_p = Wi_s[perm], Wh_s[perm], b_s[perm]
        wxa = np.zeros((EMBD + 2, 4 * H), np.float32)
        wxa[:EMBD] = Wi_p.T
        wxa[EMBD] = b_p
        if bwd:
            wxa[EMBD + 1, 0:3 * H] = -30000.0   # i, f, o gate masking
        return np.ascontiguousarray(wxa).astype(bf16), \
            np.ascontiguousarray(Wh_p.T).astype(bf16)

    wx_f, wh_f = mk(Wf_ih, Wf_hh, bfv, False)
    wx_b, wh_b = mk(Wb_ih, Wb_hh, bbv, True)

    Wg = (W_lin * gamma[None, :]) * 0.5
    wsum = (W_lin * gamma[None, :]).sum(1)
    c0 = (W_lin @ beta).astype(np.float32)
    nws = (-(wsum / 400.0)).astype(np.float32)

    kap = np.exp(-KLOG)
    mmat = np.zeros((K, K + 1), np.float32)
    mmat[:, :K] = kap * np.exp(trans)
    mmat[:, K] = np.exp(trans[:, END])

    w0 = np.zeros((K, BC), np.float32)
    w0[START, :] = 1.0

    return dict(
        wx_f=wx_f, wh_f=wh_f, wx_b=wx_b, wh_b=wh_b,
        wgt_f=np.ascontiguousarray(Wg[:, :H].T).astype(bf16),
        wgt_b=np.ascontiguousarray(Wg[:, H:].T).astype(bf16),
        negwsum=np.ascontiguousarray(nws.reshape(1, K)).astype(bf16),
        c0col=np.ascontiguousarray(c0.reshape(K, 1)),
        mmat=mmat.astype(bf16),
        w0=w0.astype(bf16),
        _emb=emb, _trans=trans, _c0=c0,
    )


def _prep_core_inputs(sent, tags, slen, consts, Tn):
    """Host-side prep for one core. sent/tags [BC,Tn] slen [BC]."""
    NT = Tn * BC
    NSC = NT // 512 // 4
    emb, trans, c0 = consts["_emb"], consts["_trans"], consts["_c0"]

    sent_tm = np.ascontiguousarray(sent.T).reshape(-1)      # t-major tokens
    uniq, inv = np.unique(sent_tm, return_inverse=True)
    embc = np.zeros((VC, EROW), np.float32)
    embc[:len(uniq), :EMBD] = emb[uniq]
    embc[:, EMBD] = 1.0
    tok16 = inv.astype(np.int16)
    gidx = np.zeros((128, NT // 16), np.int16)
    for g in range(NT // 1024):
        w = np.ascontiguousarray(tok16[g * 1024:(g + 1) * 1024].reshape(64, 16).T)
        gidx[:, g * 64:(g + 1) * 64] = np.tile(w, (8, 1))

    tgrid = np.repeat(np.arange(Tn), BC)
    bgrid = np.tile(np.arange(BC), Tn)
    invm = (tgrid >= slen[bgrid]).astype(np.float32).reshape(1, NT)

    tags_ext = np.concatenate([np.full((BC, 1), START, np.int64), tags], axis=1)
    m = (np.arange(Tn)[None, :] < slen[:, None]).astype(np.float32)  # [BC,Tn]
    mrow = (tgrid < slen[bgrid]).astype(np.float32)          # [NT] t-major
    tag_tm = tags.T.reshape(-1)
    kk = np.arange(K)[:, None]
    oh_em = ((tag_tm[None, :] == kk) * mrow[None, :]).astype(np.float32)  # [K,NT]

    # host gold terms: transition sum + END term + c0 emission part
    trans_sum = (trans[tags_ext[:, :Tn], tags_ext[:, 1:]] * m).sum(1)
    end_term = trans[tags_ext[np.arange(BC), slen], END]
    c0_sum = (c0[tags] * m).sum(1)
    lenk2 = (slen * KLOG - trans_sum - end_term - c0_sum).astype(np.float32)

    ui = (slen * BC + np.arange(BC)).astype(np.int32).reshape(BC, 1)

    d = {k: v for k, v in consts.items() if not k.startswith("_")}
    d.update(dict(
        embc=embc.astype(bf16),
        gidx=gidx,
        invm=invm.astype(bf16),
        ohem=np.ascontiguousarray(oh_em).astype(bf16),
        u_idx=ui,
        len_klog=lenk2.reshape(BC, 1),
    ))
    return d


def kernel(sentence, tags, sen_len, emb, Wf_ih, Wf_hh, bf, Wb_ih, Wb_hh, bb,
           gamma, beta, W_lin, trans):
    from concourse import bass_utils

    sentence = np.asarray(sentence).astype(np.int64)
    tags_a = np.asarray(tags).astype(np.int64)
    slen = np.asarray(sen_len).astype(np.int64)
    fp = lambda a: np.ascontiguousarray(np.asarray(a), dtype=np.float32)

    consts = _prep_consts(fp(emb), fp(Wf_ih), fp(Wf_hh), fp(bf), fp(Wb_ih), fp(Wb_hh),
                          fp(bb), fp(gamma), fp(beta), fp(W_lin), fp(trans), T)

    if T not in _PROGRAM_CACHE:
        _PROGRAM_CACHE[T] = _build_program(T)
    nc = _PROGRAM_CACHE[T]

    in_maps = []
    for core in range(NCORES):
        b0 = core * BC
        in_maps.append(_prep_core_inputs(
            sentence[b0:b0 + BC], tags_a[b0:b0 + BC], slen[b0:b0 + BC], consts, T))

    res = bass_utils.run_bass_kernel_spmd(nc, in_maps, core_ids=list(range(NCORES)))
    parts = np.concatenate([r["loss"].reshape(-1) for r in res.results])
    return np.float32(parts.mean())


if __name__ == "__main__":
    import jax
    import reference as R
    cpu = jax.devices("cpu")[0]
    with jax.default_device(cpu):
        inputs = {k: np.asarray(jax.device_put(v, cpu)) for k, v in R.setup_inputs().items()}
        expected = float(R.reference(**{k: jax.device_put(v, cpu) for k, v in inputs.items()}))
    got = kernel(**inputs)
    rel = abs(got - expected) / abs(expected)
    print("expected:", expected, "got:", got, "rel:", rel)


# revision 53
# speedup vs baseline: 1.0004x; 1.0004x over previous
"""BiLSTM-CRF loss kernel for 8 Trainium2 NeuronCores (data-parallel over batch).

Self-contained: hardcodes all shapes from the problem spec.
Returns scalar f32 loss (mean over batch of CRF NLL).

Math reformulation (validated vs reference):
 - LSTM gates via one tanh (sigmoid(x) = 0.5 tanh(x/2) + 0.5); i,f,o weight
   rows pre-halved on host. States kept as S = 2c, h' = 2h (weights absorb).
   Cell: u=(th_i+1)*th_g; w=(th_f+1)*S; S'=0.5w+u; h'=(th_o+1)*tanh(S'/2).
 - Reverse-direction masking: add -30000 to i,f,o pre-activations at padded
   steps (sigmoids -> 0 => state resets). Forward needs no masking.
 - Time-split chains: each direction's scan is split in two halves run as
   independent chains; the second half starts KW steps early from zero state
   (forget-gate decay makes the state converge; exact when the split point
   falls in padding).
 - Embedding gather via gpsimd dma_gather on a host-compacted (np.unique)
   int16-indexed table, rows padded to 128 (bias 1.0 planted at col 50),
   PE-transposed into xT.
 - LayerNorm folded into the feature matmul; mu/var from PE ones-matmul
   reductions; packing via DMA reshape.  Feats phase processed in
   super-chunks of 4x32 K-rows stacked on 128 partitions.
 - CRF in exp space: w_t = exp(alpha_t) * kappa^t with kappa folded into the
   transition matrix. alpha-at-sen_len recovered from the END-transition
   readout row, staged per step, gathered at sen_len.
 - Gold-path transition/END/beta terms are pure functions of the inputs and
   are folded into a per-sequence host constant; device computes only the
   emission sum.
"""

import numpy as np
import ml_dtypes

VOCAB, EMBD, HID, K = 100000, 50, 200, 32
H = 100
START, END = 30, 31
B, T = 512, 256
NCORES = 8
BC = B // NCORES            # 64 sequences per core
LN_EPS = 1e-5
KLOG = 4.9                  # -log(kappa)
KW = 24                     # warmup steps for second-half chains
VC = 16384                  # compacted vocab rows (>= distinct tokens/core)
EROW = 128                  # padded embedding row (bf16, 256B)

bf16 = ml_dtypes.bfloat16

_PROGRAM_CACHE = {}
DEBUG_DUMP = False


def _chain_defs(Tn):
    """Four LSTM chains (uniform length Tn//2 + KW/... ): forward split at FS,
    backward split at BS, so every chain runs exactly Tn//2 + ... slots."""
    FS = Tn // 2 + KW // 2          # 140
    BS = Tn // 2 - KW // 2          # 116
    f0 = ("f0", "f", list(range(0, FS)))                 # stores 0..FS-1
    f1 = ("f1", "f", list(range(FS - KW, Tn)))           # stores FS..Tn-1
    b0 = ("b0", "b", list(range(Tn - 1, BS - 1, -1)))    # stores BS..Tn-1
    b1 = ("b1", "b", list(range(BS - 1 + KW, -1, -1)))   # stores 0..BS-1
    chains = []
    for name, dn, ts in (f0, b0, f1, b1):
        if name == "f0":
            stored = lambda t: t < FS
        elif name == "f1":
            stored = lambda t: t >= FS
        elif name == "b0":
            stored = lambda t: t >= BS
        else:
            stored = lambda t: t < BS
        chains.append(dict(name=name, dn=dn, ts=ts, stored=stored))
    return chains


def _gather_order(Tn):
    """Order of 1024-token gather chunks (16 t-steps each) by first need."""
    FS = Tn // 2 + KW // 2
    BS = Tn // 2 - KW // 2
    ngc = Tn // 16
    def first_need(g):
        t0, t1 = 16 * g, 16 * g + 15
        cands = []
        if t0 < FS:
            cands.append(t0)                            # f0 at slot t
        if t1 >= FS - KW:
            cands.append(max(0, t0 - (FS - KW)))        # f1
        if t1 >= BS:
            cands.append(Tn - 1 - t1)                   # b0
        if t0 < BS + KW:
            cands.append(max(0, (BS - 1 + KW) - t1))    # b1
        return min(cands)
    return sorted(range(ngc), key=first_need)


def _pass1_ready(Tn):
    """Per 512-token chunk (8 t), slot at which both hf and hb are stored."""
    FS = Tn // 2 + KW // 2
    BS = Tn // 2 - KW // 2
    def ready(c):
        r = 0
        for t in range(8 * c, 8 * c + 8):
            fr = t if t < FS else t - (FS - KW)
            br = (Tn - 1 - t) if t >= BS else (BS - 1 + KW) - t
            r = max(r, fr, br)
        return r
    return [ready(c) for c in range(Tn * BC // 512)]


def _build_program(Tn):
    import concourse.bass as bass
    import concourse.bacc as bacc
    import concourse.mybir as mybir
    import concourse.tile as tile
    from concourse.alu_op_type import AluOpType as op
    from concourse.masks import make_identity
    from concourse.library_config import mlp
    from contextlib import ExitStack

    dt = mybir.dt
    AF = mybir.ActivationFunctionType
    NT = Tn * BC                 # 16384 tokens
    NGC = NT // 1024             # gather chunks
    NCH = NT // 512              # feat chunks (8 t each)
    NSC = NCH // 4               # super-chunks (32 t each)
    PKC = NT // 128              # packed free size (128)
    half = Tn // 2
    NOCT = (Tn + 1 + 7) // 8

    chains = _chain_defs(Tn)
    gorder = _gather_order(Tn)
    p1ready = _pass1_ready(Tn)
    nslots = max(len(c["ts"]) for c in chains)

    nc = bacc.Bacc()

    d_embc = nc.dram_tensor("embc", [VC, EROW], dt.bfloat16, kind="ExternalInput")
    d_gidx = nc.dram_tensor("gidx", [128, NT // 16], dt.int16, kind="ExternalInput")
    d_invm = nc.dram_tensor("invm", [1, NT], dt.bfloat16, kind="ExternalInput")
    d_wx = {dn: nc.dram_tensor(f"wx_{dn}", [EMBD + 2, 4 * H], dt.bfloat16,
                               kind="ExternalInput") for dn in "fb"}
    d_wh = {dn: nc.dram_tensor(f"wh_{dn}", [H, 4 * H], dt.bfloat16,
                               kind="ExternalInput") for dn in "fb"}
    d_wgf = nc.dram_tensor("wgt_f", [H, K], dt.bfloat16, kind="ExternalInput")
    d_wgb = nc.dram_tensor("wgt_b", [H, K], dt.bfloat16, kind="ExternalInput")
    d_nws = nc.dram_tensor("negwsum", [1, K], dt.bfloat16, kind="ExternalInput")
    d_c0 = nc.dram_tensor("c0col", [K, 1], dt.float32, kind="ExternalInput")
    d_ohem = nc.dram_tensor("ohem", [K, NT], dt.bfloat16, kind="ExternalInput")
    d_mmat = nc.dram_tensor("mmat", [K, K + 1], dt.bfloat16, kind="ExternalInput")
    d_w0 = nc.dram_tensor("w0", [K, BC], dt.bfloat16, kind="ExternalInput")
    d_ui = nc.dram_tensor("u_idx", [BC, 1], dt.int32, kind="ExternalInput")
    d_lenk = nc.dram_tensor("len_klog", [BC, 1], dt.float32, kind="ExternalInput")
    d_loss = nc.dram_tensor("loss", [BC, 1], dt.float32, kind="ExternalOutput")
    if DEBUG_DUMP:
        d_dbg = dict(
            hf=nc.dram_tensor("dbg_hf", [H, NT], dt.bfloat16, kind="ExternalOutput"),
            hb=nc.dram_tensor("dbg_hb", [H, NT], dt.bfloat16, kind="ExternalOutput"),
            xt=nc.dram_tensor("dbg_xt", [EMBD + 2, NT], dt.bfloat16, kind="ExternalOutput"),
            epk=nc.dram_tensor("dbg_epk", [K, NT], dt.bfloat16, kind="ExternalOutput"),
            mu=nc.dram_tensor("dbg_mu", [16, NT // 16], dt.float32, kind="ExternalOutput"),
            msq=nc.dram_tensor("dbg_msq", [16, NT // 16], dt.float32, kind="ExternalOutput"),
            rstd=nc.dram_tensor("dbg_rstd", [16, NT // 16], dt.bfloat16, kind="ExternalOutput"),
            rsub=nc.dram_tensor("dbg_rsub", [1, BC], dt.float32, kind="ExternalOutput"),
            ud=nc.dram_tensor("dbg_ud", [NOCT * 8 * BC, 1], dt.float32, kind="ExternalOutput"),
            ug=nc.dram_tensor("dbg_ug", [BC, 1], dt.float32, kind="ExternalOutput"),
        )

    with tile.TileContext(nc) as tc, ExitStack() as ctx:
        const = ctx.enter_context(tc.tile_pool(name="const", bufs=1))
        big = ctx.enter_context(tc.tile_pool(name="big", bufs=1))
        dramp = ctx.enter_context(tc.tile_pool(name="dramp", bufs=1, space="DRAM"))

        u_d = dramp.tile([NOCT * 8 * BC, 1], dt.float32, tag="u_d")
        r_d = dramp.tile([BC, 1], dt.float32, tag="r_d")

        nc.gpsimd.load_library(mlp)

        ident = const.tile([128, 128], dt.bfloat16)
        make_identity(nc, ident[:])
        wx = {dn: const.tile([EMBD + 2, 4 * H], dt.bfloat16, tag=f"wx{dn}", name=f"wx{dn}") for dn in "fb"}
        wh = {dn: const.tile([H, 4 * H], dt.bfloat16, tag=f"wh{dn}", name=f"wh{dn}") for dn in "fb"}
        for dn in "fb":
            nc.sync.dma_start(wx[dn][:], d_wx[dn][:])
            nc.sync.dma_start(wh[dn][:], d_wh[dn][:])
        wgf = const.tile([H, K], dt.bfloat16)
        nc.sync.dma_start(wgf[:], d_wgf[:])
        wgb = const.tile([H, K], dt.bfloat16)
        nc.sync.dma_start(wgb[:], d_wgb[:])
        nws = const.tile([1, K], dt.bfloat16)
        nc.sync.dma_start(nws[:], d_nws[:])
        c0col = const.tile([K, 1], dt.float32)
        nc.sync.dma_start(c0col[:], d_c0[:])
        ohem = const.tile([K, NT], dt.bfloat16, tag="ohem")
        nc.sync.dma_start(ohem[:], d_ohem[:])
        ones1k = const.tile([1, K], dt.bfloat16)
        nc.vector.memset(ones1k[:], 1.0)
        ones1kf = const.tile([K, 1], dt.bfloat16)
        nc.vector.memset(ones1kf[:], 1.0)
        mm_t = const.tile([K, K + 1], dt.bfloat16)
        nc.sync.dma_start(mm_t[:], d_mmat[:])
        w0t = const.tile([K, BC], dt.bfloat16)
        nc.sync.dma_start(w0t[:], d_w0[:])
        gidx = const.tile([128, NT // 16], dt.int16, tag="gidx")
        nc.sync.dma_start(gidx[:], d_gidx[:])
        ones100 = const.tile([H, 1], dt.bfloat16)
        nc.vector.memset(ones100[:], 1.0)
        ones128 = const.tile([128, 1], dt.bfloat16)
        nc.vector.memset(ones128[:], 1.0)

        xT = big.tile([128, NT], dt.bfloat16, tag="xT")
        hq = {dn: [big.tile([H, 64 * BC], dt.bfloat16, tag=f"h{dn}{q}", name=f"h{dn}{q}")
                   for q in range(Tn // 64)] for dn in "fb"}
        epkK = big.tile([K, NT], dt.bfloat16, tag="epkK")
        mspk = big.tile([128, 2 * PKC], dt.float32, tag="mspk")
        warm = {c["name"]: big.tile([H, 2 * BC], dt.bfloat16, tag=f"wm{c['name']}", name=f"wm{c['name']}")
                for c in chains if c["name"] in ("f1", "b1")}
        S = {c["name"]: [big.tile([H, BC], dt.float32, tag=f"S{c['name']}{j}", name=f"S{c['name']}{j}")
                         for j in range(2)] for c in chains}
        for c in chains:
            nc.vector.memset(S[c["name"]][0][:], 0.0)

        # ================ P0: embedding gather + transpose ================
        ctx0 = ExitStack()
        p0 = ctx0.enter_context(tc.tile_pool(name="p0", bufs=3))
        p0ps = ctx0.enter_context(tc.tile_pool(name="p0ps", bufs=1, space="PSUM"))
        for gi, g in enumerate(gorder):
            xg = p0.tile([128, 8 * EROW], dt.bfloat16, tag="xg")
            nc.gpsimd.dma_gather(
                xg[:].rearrange("p (k e) -> p k e", e=EROW),
                d_embc[:],
                gidx[:, g * 64:(g + 1) * 64],
                1024, 1024, EROW,
            )
            for hf in range(2):
                tp = p0ps.tile([EMBD + 2, 512], dt.bfloat16, tag="tp")
                for j in range(4):
                    sl = hf * 4 + j
                    nc.tensor.matmul(
                        out=tp[:, j * 128:(j + 1) * 128],
                        lhsT=xg[:, sl * EROW:sl * EROW + EMBD + 2],
                        rhs=ident[:], is_transpose=True,
                        start=(j == 0), stop=(j == 3),
                    )
                dst = g * 1024 + hf * 512
                nc.vector.tensor_copy(out=xT[0:EMBD + 2, dst:dst + 512], in_=tp[:])
            nc.sync.dma_start(out=xT[EMBD + 1:EMBD + 2, g * 1024:(g + 1) * 1024],
                              in_=d_invm[:, g * 1024:(g + 1) * 1024])
        ctx0.close()

        # ================ P1: four LSTM chains + interleaved pass1/pass2 ==
        goldps = ctx.enter_context(tc.tile_pool(name="goldps", bufs=1, space="PSUM"))
        realp = goldps.tile([1, 512], dt.float32, tag="realp")
        ctx1 = ExitStack()
        p1 = ctx1.enter_context(tc.tile_pool(name="p1", bufs=2))
        p1psA = ctx1.enter_context(tc.tile_pool(name="p1psA", bufs=1, space="PSUM"))
        p1psB = ctx1.enter_context(tc.tile_pool(name="p1psB", bufs=1, space="PSUM"))
        p2a = ctx1.enter_context(tc.tile_pool(name="p2a", bufs=2))
        p2aps = ctx1.enter_context(tc.tile_pool(name="p2aps", bufs=1, space="PSUM"))
        p2 = ctx1.enter_context(tc.tile_pool(name="p2", bufs=3))
        p2s = ctx1.enter_context(tc.tile_pool(name="p2s", bufs=1))
        p2ps = ctx1.enter_context(tc.tile_pool(name="p2ps", bufs=1, space="PSUM"))
        packRS = p2s.tile([128, 2 * PKC], dt.bfloat16, tag="packRS")
        sq = p2s.tile([128, PKC], dt.float32, tag="sqpk")
        var = p2s.tile([128, PKC], dt.float32, tag="varpk")
        lnv = p2s.tile([128, PKC], dt.float32, tag="lnvpk")
        epsc = p2s.tile([128, 1], dt.float32, tag="epsc")
        nc.vector.memset(epsc[:], LN_EPS)

        def h_ap(c, j):
            """AP where chain c's step-j h output lives."""
            t = c["ts"][j]
            if c["stored"](t):
                return hq[c["dn"]][t // 64][:, (t % 64) * BC:(t % 64 + 1) * BC]
            return warm[c["name"]][:, (j % 2) * BC:(j % 2 + 1) * BC]

        p1_stash = {}

        def emit_pass1(ch):
            q, off = (ch * 512) // (64 * BC), (ch * 512) % (64 * BC)
            hfc = hq["f"][q][:, off:off + 512]
            hbc = hq["b"][q][:, off:off + 512]
            hsqf = p2a.tile([H, 512], dt.bfloat16, tag="hsqf")
            nc.gpsimd.tensor_tensor(out=hsqf[:], in0=hfc, in1=hfc, op=op.mult)
            hsqb = p2a.tile([H, 512], dt.bfloat16, tag="hsqb")
            nc.gpsimd.tensor_tensor(out=hsqb[:], in0=hbc, in1=hbc, op=op.mult)
            stgb = p2a.tile([1, 1024], dt.float32, tag="stgb")
            psmu = p2aps.tile([1, 512], dt.float32, tag="psmu")
            nc.tensor.matmul(out=psmu[:], lhsT=ones100[:], rhs=hfc, start=True, stop=False)
            nc.tensor.matmul(out=psmu[:], lhsT=ones100[:], rhs=hbc, start=False, stop=True)
            nc.vector.tensor_copy(out=stgb[:, 0:512], in_=psmu[:])
            psmsq = p2aps.tile([1, 512], dt.float32, tag="psmu")
            nc.tensor.matmul(out=psmsq[:], lhsT=ones100[:], rhs=hsqf[:], start=True, stop=False)
            nc.tensor.matmul(out=psmsq[:], lhsT=ones100[:], rhs=hsqb[:], start=False, stop=True)
            nc.vector.tensor_copy(out=stgb[:, 512:1024], in_=psmsq[:])
            nc.sync.dma_start(out=mspk[4 * ch:4 * ch + 4, 0:PKC],
                              in_=stgb[:, 0:512])
            nc.sync.dma_start(out=mspk[4 * ch:4 * ch + 4, PKC:2 * PKC],
                              in_=stgb[:, 512:1024])

        def emit_rstd(g):
            r = slice(32 * g, 32 * g + 32)
            nc.vector.scalar_tensor_tensor(out=sq[r, :], in0=mspk[r, 0:PKC],
                                           scalar=1.0 / 160000.0,
                                           in1=mspk[r, 0:PKC], op0=op.mult, op1=op.mult)
            nc.vector.scalar_tensor_tensor(out=var[r, :], in0=mspk[r, PKC:2 * PKC],
                                           scalar=1.0 / 800.0,
                                           in1=sq[r, :], op0=op.mult, op1=op.subtract)
            nc.scalar.activation(out=lnv[r, :], in_=var[r, :], func=AF.Ln,
                                 bias=epsc[r, :])
            nc.scalar.activation(out=packRS[r, 0:PKC], in_=lnv[r, :], func=AF.Exp,
                                 scale=-0.5)
            nc.vector.tensor_copy(out=packRS[r, PKC:2 * PKC], in_=mspk[r, 0:PKC])

        n_p2 = [0]

        def emit_pass2(ch):
            q, off = (ch * 512) // (64 * BC), (ch * 512) % (64 * BC)
            pg = p2ps.tile([K, 512], dt.float32, tag="pg")
            uboth = p2.tile([1, 1024], dt.bfloat16, tag="uboth")
            nc.sync.dma_start(out=uboth[:, 0:512],
                              in_=packRS[4 * ch:4 * ch + 4, 0:PKC])
            nc.sync.dma_start(out=uboth[:, 512:1024],
                              in_=packRS[4 * ch:4 * ch + 4, PKC:2 * PKC])
            nc.tensor.matmul(out=pg[:], lhsT=wgf[:], rhs=hq["f"][q][:, off:off + 512],
                             start=True, stop=False)
            nc.tensor.matmul(out=pg[:], lhsT=wgb[:], rhs=hq["b"][q][:, off:off + 512],
                             start=False, stop=False)
            nc.tensor.matmul(out=pg[:], lhsT=nws[:], rhs=uboth[:, 512:1024],
                             start=False, stop=True)
            rb = p2ps.tile([K, 512], dt.float32, tag="rb")
            nc.tensor.matmul(out=rb[:], lhsT=ones1k[:], rhs=uboth[:, 0:512],
                             start=True, stop=True)
            rbs = p2.tile([K, 512], dt.bfloat16, tag="rbs")
            nc.scalar.copy(out=rbs[:], in_=rb[:])
            fsl = p2.tile([K, 512], dt.bfloat16, tag="fsl")
            nc.vector.tensor_tensor(out=fsl[:], in0=pg[:], in1=rbs[:], op=op.mult)
            nc.scalar.activation(out=epkK[:, ch * 512:(ch + 1) * 512], in_=fsl[:],
                                 func=AF.Exp, bias=c0col[:])
            esel = p2.tile([K, 512], dt.bfloat16, tag="esel")
            nc.vector.tensor_tensor(out=esel[:], in0=fsl[:],
                                    in1=ohem[:, ch * 512:(ch + 1) * 512],
                                    op=op.mult)
            k = n_p2[0]
            nc.tensor.matmul(out=realp[:], lhsT=ones1kf[:], rhs=esel[:],
                             start=(k == 0), stop=(k == NCH - 1))
            n_p2[0] += 1

        grp_ready = [max(p1ready[8 * g:8 * g + 8]) for g in range(NCH // 8)]
        pending1 = {s: [c for c in range(NCH) if p1ready[c] == s] for s in range(nslots)}
        pending2 = {s: [g for g in range(NCH // 8) if grp_ready[g] == s]
                    for s in range(nslots)}

        for s in range(nslots):
            act_cs = [c for c in chains if s < len(c["ts"])]
            step = {}
            for c in act_cs:
                nm, dn = c["name"], c["dn"]
                t = c["ts"][s]
                first = (s == 0)
                pspool = p1psB if nm in ("f1", "b1") else p1psA
                ps = pspool.tile([H, 4 * BC], dt.float32, tag=f"ps{nm}", name=f"ps{nm}")
                n_mm = 4 if first else 8
                k_mm = 0
                rx = xT[0:EMBD + 2, t * BC:(t + 1) * BC]
                for g in range(4):
                    nc.tensor.matmul(out=ps[:, g * BC:(g + 1) * BC],
                                     lhsT=wx[dn][:, g * H:(g + 1) * H], rhs=rx,
                                     start=(k_mm == 0), stop=(k_mm == n_mm - 1))
                    k_mm += 1
                if not first:
                    rh = h_ap(c, s - 1)
                    for g in range(4):
                        nc.tensor.matmul(out=ps[:, g * BC:(g + 1) * BC],
                                         lhsT=wh[dn][:, g * H:(g + 1) * H], rhs=rh,
                                         start=(k_mm == 0), stop=(k_mm == n_mm - 1))
                        k_mm += 1
                step[nm] = ps
            for c in act_cs:
                nm = c["name"]
                G = p1.tile([H, 4 * BC], dt.bfloat16, tag=f"G{nm}", name=f"G{nm}")
                nc.scalar.activation(out=G[:], in_=step[nm][:], func=AF.Tanh)
                step[nm] = G
            for c in act_cs:
                nm = c["name"]
                G = step[nm]
                u = p1.tile([H, BC], dt.bfloat16, tag=f"u{nm}", name=f"u{nm}")
                nc.vector.scalar_tensor_tensor(out=u[:], in0=G[:, 0:BC], scalar=1.0,
                                               in1=G[:, 3 * BC:4 * BC],
                                               op0=op.add, op1=op.mult)
                w = p1.tile([H, BC], dt.float32, tag=f"w{nm}", name=f"w{nm}")
                nc.vector.scalar_tensor_tensor(out=w[:], in0=G[:, BC:2 * BC], scalar=1.0,
                                               in1=S[nm][s % 2][:],
                                               op0=op.add, op1=op.mult)
                nc.vector.scalar_tensor_tensor(out=S[nm][(s + 1) % 2][:], in0=w[:],
                                               scalar=0.5, in1=u[:],
                                               op0=op.mult, op1=op.add)
            thcs = {}
            for c in act_cs:
                nm = c["name"]
                thc = p1.tile([H, BC], dt.bfloat16, tag=f"thc{nm}", name=f"thc{nm}")
                nc.scalar.activation(out=thc[:], in_=S[nm][(s + 1) % 2][:],
                                     func=AF.Tanh, scale=0.5)
                thcs[nm] = thc
            for c in act_cs:
                nm = c["name"]
                nc.vector.scalar_tensor_tensor(out=h_ap(c, s),
                                               in0=step[nm][:, 2 * BC:3 * BC],
                                               scalar=1.0, in1=thcs[nm][:],
                                               op0=op.add, op1=op.mult)
            for ch in pending1.get(s, []):
                emit_pass1(ch)
            for g in pending2.get(s, []):
                emit_rstd(g)
                for ch in range(8 * g, 8 * g + 8):
                    emit_pass2(ch)
        if DEBUG_DUMP:
            nc.sync.dma_start(d_dbg["rstd"][:], packRS[:, 0:PKC])
        ctx1.close()

        # ================ P3: CRF recursion ===============================
        with tc.tile_pool(name="p3", bufs=2) as p3, \
             tc.tile_pool(name="p3ps", bufs=4, space="PSUM") as p3ps:
            wcur = w0t
            for o in range(NOCT):
                t0, t1 = o * 8 + 1, min(o * 8 + 8, Tn + 1)
                nsteps = t1 - t0 + 1
                pvo = p3ps.tile([K + 1, 512], dt.float32, tag="pvo")
                ustg = p3.tile([K + 1, 512], dt.float32, tag="ustg")
                for t in range(t0, t1 + 1):
                    so = (t - 1) % 8
                    pv = pvo[:, so * BC:(so + 1) * BC]
                    nc.tensor.matmul(out=pv[:], lhsT=mm_t[:],
                                     rhs=wcur[:], start=True, stop=True)
                    if t <= Tn:
                        tok = t - 1
                        wn = p3.tile([K, BC], dt.bfloat16, tag="wn")
                        nc.vector.tensor_tensor(
                            out=wn[:], in0=pv[0:K, :],
                            in1=epkK[:, tok * BC:(tok + 1) * BC],
                            op=op.mult)
                        wcur = wn
                nc.scalar.copy(out=ustg[K:K + 1, :nsteps * BC],
                               in_=pvo[K:K + 1, :nsteps * BC])
                nc.sync.dma_start(
                    out=u_d[(t0 - 1) * BC:(t0 - 1) * BC + nsteps * BC, :],
                    in_=ustg[K:K + 1, :nsteps * BC])

        # ================ P4: final loss ==================================
        with tc.tile_pool(name="p4", bufs=1) as p4, \
             tc.tile_pool(name="p4ps", bufs=1, space="PSUM") as p4ps:
            rsub = p4.tile([1, BC], dt.float32, tag="rsub")
            nc.vector.tensor_reduce(
                out=rsub[:], in_=realp[:].rearrange("one (t b) -> one b t", b=BC),
                axis=mybir.AxisListType.X, op=op.add)
            if DEBUG_DUMP:
                nc.sync.dma_start(d_dbg["rsub"][:], rsub[:])
            rsubh = p4.tile([1, BC], dt.bfloat16, tag="rsubh")
            nc.vector.tensor_copy(out=rsubh[:], in_=rsub[:])
            rcps = p4ps.tile([BC, 1], dt.bfloat16, tag="rcps")
            nc.tensor.matmul(out=rcps[:], lhsT=rsubh[:], rhs=ident[0:1, 0:1],
                             is_transpose=True, start=True, stop=True)
            rcol = p4.tile([BC, 1], dt.float32, tag="rcol")
            nc.vector.tensor_copy(out=rcol[:], in_=rcps[:])

            ui = p4.tile([BC, 1], dt.int32, tag="ui")
            nc.sync.dma_start(ui[:], d_ui[:])
            lenk = p4.tile([BC, 1], dt.float32, tag="lenk")
            nc.sync.dma_start(lenk[:], d_lenk[:])
            ug = p4.tile([BC, 1], dt.float32, tag="ug")
            nc.gpsimd.indirect_dma_start(out=ug[:], out_offset=None, in_=u_d[:],
                                         in_offset=bass.IndirectOffsetOnAxis(ap=ui[:], axis=0))
            tot = p4.tile([BC, 1], dt.float32, tag="tot")
            if DEBUG_DUMP:
                nc.sync.dma_start(d_dbg["ug"][:], ug[:])
            nc.scalar.activation(out=tot[:], in_=ug[:], func=AF.Ln)
            nc.vector.tensor_tensor(out=tot[:], in0=tot[:], in1=lenk[:], op=op.add)
            lout = p4.tile([BC, 1], dt.float32, tag="lout")
            nc.vector.tensor_tensor(out=lout[:], in0=tot[:], in1=rcol[:], op=op.subtract)
            nc.sync.dma_start(out=d_loss[:], in_=lout[:])

    nc.compile()
    return nc


def _prep_consts(emb, Wf_ih, Wf_hh, bfv, Wb_ih, Wb_hh, bbv, gamma, beta, W_lin, trans, Tn):
    sc = np.ones((4 * H, 1), np.float32)
    sc[0:H] = 0.5
    sc[H:2 * H] = 0.5
    sc[3 * H:4 * H] = 0.5
    # reference gate order [i,f,g,o] -> device order [i,f,o,g]
    perm = np.concatenate([np.arange(0, H), np.arange(H, 2 * H),
                           np.arange(3 * H, 4 * H), np.arange(2 * H, 3 * H)])

    def mk(Wi, Wh, b, bwd):
        Wi_s, Wh_s, b_s = Wi * sc, Wh * sc * 0.5, b * sc[:, 0]
        Wi_p, Wh_p, b_p = Wi_s[perm], Wh_s[perm], b_s[perm]
        wxa = np.zeros((EMBD + 2, 4 * H), np.float32)
        wxa[:EMBD] = Wi_p.T
        wxa[EMBD] = b_p
        if bwd:
            wxa[EMBD + 1, 0:3 * H] = -30000.0   # i, f, o gate masking
        return np.ascontiguousarray(wxa).astype(bf16), \
            np.ascontiguousarray(Wh_p.T).astype(bf16)

    wx_f, wh_f = mk(Wf_ih, Wf_hh, bfv, False)
    wx_b, wh_b = mk(Wb_ih, Wb_hh, bbv, True)

    Wg = (W_lin * gamma[None, :]) * 0.5
    wsum = (W_lin * gamma[None, :]).sum(1)
    c0 = (W_lin @ beta).astype(np.float32)
    nws = (-(wsum / 400.0)).astype(np.float32)

    kap = np.exp(-KLOG)
    mmat = np.zeros((K, K + 1), np.float32)
    mmat[:, :K] = kap * np.exp(trans)
    mmat[:, K] = np.exp(trans[:, END])

    w0 = np.zeros((K, BC), np.float32)
    w0[START, :] = 1.0

    return dict(
        wx_f=wx_f, wh_f=wh_f, wx_b=wx_b, wh_b=wh_b,
        wgt_f=np.ascontiguousarray(Wg[:, :H].T).astype(bf16),
        wgt_b=np.ascontiguousarray(Wg[:, H:].T).astype(bf16),
        negwsum=np.ascontiguousarray(nws.reshape(1, K)).astype(bf16),
        c0col=np.ascontiguousarray(c0.reshape(K, 1)),
        mmat=mmat.astype(bf16),
        w0=w0.astype(bf16),
        _emb=emb, _trans=trans, _c0=c0,
    )


def _prep_core_inputs(sent, tags, slen, consts, Tn):
    """Host-side prep for one core. sent/tags [BC,Tn] slen [BC]."""
    NT = Tn * BC
    NSC = NT // 512 // 4
    emb, trans, c0 = consts["_emb"], consts["_trans"], consts["_c0"]

    sent_tm = np.ascontiguousarray(sent.T).reshape(-1)      # t-major tokens
    uniq, inv = np.unique(sent_tm, return_inverse=True)
    embc = np.zeros((VC, EROW), np.float32)
    embc[:len(uniq), :EMBD] = emb[uniq]
    embc[:, EMBD] = 1.0
    tok16 = inv.astype(np.int16)
    gidx = np.zeros((128, NT // 16), np.int16)
    for g in range(NT // 1024):
        w = np.ascontiguousarray(tok16[g * 1024:(g + 1) * 1024].reshape(64, 16).T)
        gidx[:, g * 64:(g + 1) * 64] = np.tile(w, (8, 1))

    tgrid = np.repeat(np.arange(Tn), BC)
    bgrid = np.tile(np.arange(BC), Tn)
    invm = (tgrid >= slen[bgrid]).astype(np.float32).reshape(1, NT)

    tags_ext = np.concatenate([np.full((BC, 1), START, np.int64), tags], axis=1)
    m = (np.arange(Tn)[None, :] < slen[:, None]).astype(np.float32)  # [BC,Tn]
    mrow = (tgrid < slen[bgrid]).astype(np.float32)          # [NT] t-major
    tag_tm = tags.T.reshape(-1)
    kk = np.arange(K)[:, None]
    oh_em = ((tag_tm[None, :] == kk) * mrow[None, :]).astype(np.float32)  # [K,NT]

    # host gold terms: transition sum + END term + c0 emission part
    trans_sum = (trans[tags_ext[:, :Tn], tags_ext[:, 1:]] * m).sum(1)
    end_term = trans[tags_ext[np.arange(BC), slen], END]
    c0_sum = (c0[tags] * m).sum(1)
    lenk2 = (slen * KLOG - trans_sum - end_term - c0_sum).astype(np.float32)

    ui = (slen * BC + np.arange(BC)).astype(np.int32).reshape(BC, 1)

    d = {k: v for k, v in consts.items() if not k.startswith("_")}
    d.update(dict(
        embc=embc.astype(bf16),
        gidx=gidx,
        invm=invm.astype(bf16),
        ohem=np.ascontiguousarray(oh_em).astype(bf16),
        u_idx=ui,
        len_klog=lenk2.reshape(BC, 1),
    ))
    return d


def kernel(sentence, tags, sen_len, emb, Wf_ih, Wf_hh, bf, Wb_ih, Wb_hh, bb,
           gamma, beta, W_lin, trans):
    from concourse import bass_utils

    sentence = np.asarray(sentence).astype(np.int64)
    tags_a = np.asarray(tags).astype(np.int64)
    slen = np.asarray(sen_len).astype(np.int64)
    fp = lambda a: np.ascontiguousarray(np.asarray(a), dtype=np.float32)

    consts = _prep_consts(fp(emb), fp(Wf_ih), fp(Wf_hh), fp(bf), fp(Wb_ih), fp(Wb_hh),
                          fp(bb), fp(gamma), fp(beta), fp(W_lin), fp(trans), T)

    if T not in _PROGRAM_CACHE:
        _PROGRAM_CACHE[T] = _build_program(T)
    nc = _PROGRAM_CACHE[T]

    in_maps = []
    for core in range(NCORES):
        b0 = core * BC
        in_maps.append(_prep_core_inputs(
            sentence[b0:b0 + BC], tags_a[b0:b0 + BC], slen[b0:b0 + BC], consts, T))

    res = bass_utils.run_bass_kernel_spmd(nc, in_maps, core_ids=list(range(NCORES)))
    parts = np.concatenate([r["loss"].reshape(-1) for r in res.results])
    return np.float32(parts.mean())


if __name__ == "__main__":
    import jax
    import reference as R
    cpu = jax.devices("cpu")[0]
    with jax.default_device(cpu):
        inputs = {k: np.asarray(jax.device_put(v, cpu)) for k, v in R.setup_inputs().items()}
        expected = float(R.reference(**{k: jax.device_put(v, cpu) for k, v in inputs.items()}))
    got = kernel(**inputs)
    rel = abs(got - expected) / abs(expected)
    print("expected:", expected, "got:", got, "rel:", rel)
